# revision 1
# baseline (speedup 1.0000x reference)
"""Trainium2 Bass kernel for nn_MHA_43095701848407.

MHA forward: qkv = x @ W_qkv, RoPE on q/k, causal softmax attention,
y @ W_proj.  B=4, T=2048, C=2048, 16 heads, head_dim=128, fp32.

Sharding (8 cores): tensor-parallel over heads (4 shards x 4 heads) x
data-parallel over batch (2 groups x 2 batches).  core = group*4 + shard.
Each core computes, for its 2 batches and 4 heads:
  qkv^T tiles via fp32r matmuls (x^T streamed, W resident),
  RoPE via a permutation matmul + vector combines,
  causal attention in transposed orientation (scores^T [k,q]; exp on ACT;
  column sums via ones-matmul; y^T = v_nat.T @ p^T), then the local slice
  of the output projection, producing a partial out^T [C, T] per batch.
Host sums the 4 head-shard partials per batch and transposes back.

Self-contained: shapes/sharding hardcoded; inputs full-size numpy arrays.
"""

import math
import os
import sys
import types

import numpy as np

import concourse.bass as bass
import concourse.mybir as mybir
import concourse.tile as tile
from concourse import bacc
from concourse.bass_utils import run_bass_kernel_spmd

F32 = mybir.dt.float32
F32R = mybir.dt.float32r
AF = mybir.ActivationFunctionType
ALU = mybir.AluOpType

# Problem shape (hardcoded per contract)
B, T, C = 4, 2048, 2048
H, HD = 16, 128
NCORES = 8
BGROUPS, HSHARDS = 2, 4  # batch groups x head shards
B_LOC = B // BGROUPS  # 2 batches per core
H_LOC = H // HSHARDS  # 4 heads per core
FQK = H_LOC * HD  # 512 features for q (and for k)
FV = H_LOC * HD  # 512 features for v
F_ALL = 3 * H_LOC * HD  # 1536 qkv features per core
KO = C // 128  # 16 contraction chunks
TSLAB = 512
NSLAB = T // TSLAB  # 4 t-slabs per batch
QTILE = 512
NQT = T // QTILE  # 4 q-tiles
NKB = T // 128  # 16 key blocks
SCALE = 1.0 / math.sqrt(HD)

_CACHED = {}


def _install_ntff_hook():
    """Register the axon NTFF profile hook (container's antenv lacks it)."""
    if "antenv.axon_hooks" in sys.modules:
        return
    try:
        mod = types.ModuleType("antenv.axon_hooks")
        holder = [None]
        mod.set_axon_ntff_profile_hook = lambda h: holder.__setitem__(0, h)
        mod.get_axon_ntff_profile_hook = lambda: holder[0]
        sys.modules["antenv.axon_hooks"] = mod
        import antenv

        antenv.axon_hooks = mod
        if "/root/.axon_site" not in sys.path:
            sys.path.insert(0, "/root/.axon_site")
        from trn_agent_boot.trn_boot import _ntff_profile_via_ctypes

        mod.set_axon_ntff_profile_hook(
            _ntff_profile_via_ctypes("/opt/axon/libaxon_pjrt.so")
        )
    except Exception:
        sys.modules.pop("antenv.axon_hooks", None)


def rope_perm_matrix():
    """lhsT for the rotate-half matmul: rot^T = PT.T @ q^T.
    rot[2i] = -q[2i+1], rot[2i+1] = q[2i]."""
    pt = np.zeros((HD, HD), dtype=np.float32)
    for i in range(HD // 2):
        pt[2 * i + 1, 2 * i] = -1.0
        pt[2 * i, 2 * i + 1] = 1.0
    return pt


def build_nc():
    nc = bacc.Bacc("TRN2", target_bir_lowering=False, debug=False)

    x_t = nc.dram_tensor("x_t", [B_LOC, C, T], F32R, kind="ExternalInput").ap()
    w_qkv = nc.dram_tensor("w_qkv", [C, F_ALL], F32R, kind="ExternalInput").ap()
    w_proj = nc.dram_tensor("w_proj", [FV, C], F32R, kind="ExternalInput").ap()
    sin_t = nc.dram_tensor("sin_t", [HD, T], F32, kind="ExternalInput").ap()
    cos_t = nc.dram_tensor("cos_t", [HD, T], F32, kind="ExternalInput").ap()
    pt = nc.dram_tensor("pt", [HD, HD], F32R, kind="ExternalInput").ap()
    ones_col = nc.dram_tensor("ones_col", [128, 1], F32R, kind="ExternalInput").ap()
    ones_row = nc.dram_tensor("ones_row", [1, 128], F32R, kind="ExternalInput").ap()
    out_t = nc.dram_tensor("out_t", [B_LOC, C, T], F32, kind="ExternalOutput").ap()

    with tile.TileContext(nc) as tc:
        with nc.allow_low_precision(reason="fp32r matmul inputs by design"):
            _emit(nc, tc, x_t, w_qkv, w_proj, sin_t, cos_t, pt, ones_col,
                  ones_row, out_t)
    nc.compile()
    return nc


def _emit(nc, tc, x_t, w_qkv, w_proj, sin_t, cos_t, pt, ones_col, ones_row, out_t):
    # ---- persistent scratch in DRAM ----
    with tc.tile_pool(name="dram", bufs=1, space="DRAM") as dram_pool:
        qk_dram = [
            dram_pool.tile([2 * FQK, T], F32R, name=f"qk_dram{b}") for b in range(B_LOC)
        ]
        v_dram = [
            dram_pool.tile([T, FV], F32R, name=f"v_dram{b}") for b in range(B_LOC)
        ]

        with tc.tile_pool(name="consts", bufs=1) as consts:
            pt_sb = consts.tile([HD, HD], F32R)
            nc.sync.dma_start(pt_sb, pt)
            ones_c_sb = consts.tile([128, 1], F32R)
            nc.sync.dma_start(ones_c_sb, ones_col)
            ones_r_sb = consts.tile([1, 128], F32R)
            nc.sync.dma_start(ones_r_sb, ones_row)

            _phase_qkv(nc, tc, x_t, w_qkv, sin_t, cos_t, pt_sb, qk_dram, v_dram)
            _phase_attn_proj(
                nc, tc, w_proj, qk_dram, v_dram, ones_c_sb, ones_r_sb, out_t
            )


def _phase_qkv(nc, tc, x_t, w_qkv, sin_t, cos_t, pt_sb, qk_dram, v_dram):
    """qkv^T = W.T @ x^T with RoPE on q,k; v in natural [t, f] layout."""
    with (
        tc.tile_pool(name="wpool", bufs=1) as wpool,
        tc.tile_pool(name="xpool", bufs=2) as xpool,
        tc.tile_pool(name="scpool", bufs=2) as scpool,
        tc.tile_pool(name="ropepool", bufs=2) as ropepool,
        tc.tile_pool(name="qkpsum", bufs=3, space="PSUM") as qkpsum,
        tc.tile_pool(name="rotpsum", bufs=2, space="PSUM") as rotpsum,
        tc.tile_pool(name="vpsum", bufs=2, space="PSUM") as vpsum,
    ):
        w_sb = wpool.tile([128, KO, F_ALL], F32R)
        w_src = w_qkv.rearrange("(ko p) f -> p ko f", p=128)
        for ko in range(KO):
            nc.scalar.dma_start(w_sb[:, ko, :], w_src[:, ko, :])

        for b in range(B_LOC):
            x3 = x_t[b].rearrange("(ko p) t -> p ko t", p=128)
            for js in range(NSLAB):
                first = b == 0 and js == 0
                tsl = slice(js * TSLAB, (js + 1) * TSLAB)
                x_sb = xpool.tile([128, KO, TSLAB], F32R, name="x_sb")
                if first:
                    # split by ko so the first matmuls start after ~1/16 load
                    for ko in range(KO):
                        nc.sync.dma_start(x_sb[:, ko, :], x3[:, ko, tsl])
                else:
                    nc.sync.dma_start(x_sb, x3[:, :, tsl])
                sin_sb = scpool.tile([HD, TSLAB], F32, name="sin_sb")
                nc.sync.dma_start(sin_sb, sin_t[:, tsl])
                cos_sb = scpool.tile([HD, TSLAB], F32, name="cos_sb")
                nc.sync.dma_start(cos_sb, cos_t[:, tsl])

                # q^T, k^T feature chunks (heads) with RoPE
                qk_psums = {}
                if first:
                    # ko-outer in two groups of 4 f-chunks: compute proceeds at
                    # W/x chunk-arrival pace instead of waiting for full load
                    for fg in range(2):
                        fs = [fg * 4 + i for i in range(4)]
                        pss = {
                            f: qkpsum.tile([128, TSLAB], F32, name="qk_ps")
                            for f in fs
                        }
                        for ko in range(KO):
                            for f in fs:
                                nc.tensor.matmul(
                                    pss[f],
                                    w_sb[:, ko, f * 128 : (f + 1) * 128],
                                    x_sb[:, ko, :],
                                    start=(ko == 0),
                                    stop=(ko == KO - 1),
                                )
                        qk_psums.update(pss)
                for f in range(2 * H_LOC):
                    if first:
                        ps = qk_psums[f]
                    else:
                        ps = qkpsum.tile([128, TSLAB], F32, name="qk_ps")
                        for ko in range(KO):
                            nc.tensor.matmul(
                                ps,
                                w_sb[:, ko, f * 128 : (f + 1) * 128],
                                x_sb[:, ko, :],
                                start=(ko == 0),
                                stop=(ko == KO - 1),
                            )
                    raw = ropepool.tile([128, TSLAB], F32R, name="raw")
                    nc.vector.tensor_copy(raw, ps)
                    rot_ps = rotpsum.tile([128, TSLAB], F32, name="rot_ps")
                    nc.tensor.matmul(rot_ps, pt_sb, raw, start=True, stop=True)
                    # roped = raw*cos + rot*sin
                    t1 = ropepool.tile([128, TSLAB], F32, name="t1")
                    nc.gpsimd.tensor_tensor(t1, raw, cos_sb, ALU.mult)
                    t2 = ropepool.tile([128, TSLAB], F32, name="t2")
                    nc.vector.tensor_tensor(t2, rot_ps, sin_sb, ALU.mult)
                    roped = ropepool.tile([128, TSLAB], F32R, name="roped")
                    nc.vector.tensor_tensor(roped, t1, t2, ALU.add)
                    nc.sync.dma_start(
                        qk_dram[b][f * 128 : (f + 1) * 128, tsl], roped
                    )

                # v in natural layout
                for tb in range(TSLAB // 128):
                    vps = vpsum.tile([128, FV], F32, name="v_ps")
                    for ko in range(KO):
                        nc.tensor.matmul(
                            vps,
                            x_sb[:, ko, tb * 128 : (tb + 1) * 128],
                            w_sb[:, ko, 2 * FQK : 2 * FQK + FV],
                            start=(ko == 0),
                            stop=(ko == KO - 1),
                        )
                    v_sb = ropepool.tile([128, FV], F32R, name="v_sb")
                    nc.vector.tensor_copy(v_sb, vps)
                    r0 = js * TSLAB + tb * 128
                    nc.sync.dma_start(v_dram[b][r0 : r0 + 128, :], v_sb)


def _phase_attn_proj(nc, tc, w_proj, qk_dram, v_dram, ones_c_sb, ones_r_sb, out_t):
    with (
        tc.tile_pool(name="wppool", bufs=1) as wppool,
        tc.tile_pool(name="qkvload", bufs=3) as qkvload,
        tc.tile_pool(name="ppool", bufs=6) as ppool,
        tc.tile_pool(name="ypool", bufs=B_LOC * H_LOC) as ypool,
        tc.tile_pool(name="npool", bufs=5) as npool,
        tc.tile_pool(name="opool", bufs=3) as opool,
        tc.tile_pool(name="spsum", bufs=3, space="PSUM") as spsum,
        tc.tile_pool(name="ypsum", bufs=2, space="PSUM") as ypsum,
        tc.tile_pool(name="lpsum", bufs=1, space="PSUM") as lpsum,
        tc.tile_pool(name="opsum", bufs=2, space="PSUM") as opsum,
        tc.tile_pool(name="nbounce", bufs=4, space="DRAM") as nbounce,
    ):
        wp_sb = wppool.tile([128, H_LOC, C], F32R)
        nc.sync.dma_start(wp_sb, w_proj.rearrange("(fo p) c -> p fo c", p=128))

        def emit_head_load(b, h):
            qt_sb = qkvload.tile([HD, T], F32R, name="qt_sb")
            nc.scalar.dma_start(qt_sb, qk_dram[b][h * HD : (h + 1) * HD, :])
            kt_sb = qkvload.tile([HD, T], F32R, name="kt_sb")
            nc.scalar.dma_start(
                kt_sb, qk_dram[b][FQK + h * HD : FQK + (h + 1) * HD, :]
            )
            v_sb = qkvload.tile([128, NKB, HD], F32R, name="v_sb")
            nc.scalar.dma_start(
                v_sb,
                v_dram[b].rearrange("(kb p) f -> p kb f", p=128)[
                    :, :, h * HD : (h + 1) * HD
                ],
            )
            return qt_sb, kt_sb, v_sb

        bh_pairs = [(b, h) for b in range(B_LOC) for h in range(H_LOC)]
        pending = {}
        pending[bh_pairs[0]] = emit_head_load(*bh_pairs[0])

        y_by_batch = {b: [] for b in range(B_LOC)}
        for bh_i, (b, h) in enumerate(bh_pairs):
            y_tiles = y_by_batch[b]
            if True:
                if bh_i + 1 < len(bh_pairs):
                    pending[bh_pairs[bh_i + 1]] = emit_head_load(*bh_pairs[bh_i + 1])
                qt_sb, kt_sb, v_sb = pending.pop((b, h))
                y_sb = ypool.tile([HD, T], F32R, name="y_sb")
                y_tiles.append(y_sb)

                norm_pairs = []
                for jq in range(NQT - 1, -1, -1):
                    qsl = slice(jq * QTILE, (jq + 1) * QTILE)
                    nkb = 4 * (jq + 1)
                    y_ps = ypsum.tile([HD, QTILE], F32, name="y_ps")
                    l_ps = lpsum.tile([1, QTILE], F32, name="l_ps")
                    for kb in range(nkb):
                        # diagonal blocks only touch q >= qoff within this tile
                        s_diag = kb - 4 * jq
                        qoff = 128 * s_diag if s_diag > 0 else 0
                        qn = QTILE - qoff
                        qsub = slice(jq * QTILE + qoff, (jq + 1) * QTILE)
                        s_ps = spsum.tile([128, QTILE], F32, name="s_ps")
                        nc.tensor.matmul(
                            s_ps[:, qoff:],
                            kt_sb[:, kb * 128 : (kb + 1) * 128],
                            qt_sb[:, qsub],
                            start=True,
                            stop=True,
                        )
                        p_sb = ppool.tile([128, QTILE], F32R, name="p_sb")
                        nc.scalar.activation(
                            p_sb[:, qoff:], s_ps[:, qoff:], AF.Exp, scale=SCALE
                        )
                        if s_diag >= 0:
                            # causal: keep where (q - qoff) - k >= 0 in sub-range
                            nc.gpsimd.affine_select(
                                out=p_sb[:, qoff:],
                                in_=p_sb[:, qoff:],
                                pattern=[[1, qn]],
                                compare_op=ALU.is_ge,
                                fill=0.0,
                                base=0,
                                channel_multiplier=-1,
                            )
                        nc.tensor.matmul(
                            l_ps[:, qoff:],
                            ones_c_sb,
                            p_sb[:, qoff:],
                            start=(kb == 0),
                            stop=(kb == nkb - 1),
                        )
                        nc.tensor.matmul(
                            y_ps[:, qoff:],
                            v_sb[:, kb, :],
                            p_sb[:, qoff:],
                            start=(kb == 0),
                            stop=(kb == nkb - 1),
                        )
                    # evacuate y unnormalized immediately (frees the psum bank),
                    # then normalize in place once 1/l is broadcast
                    nc.vector.tensor_copy(y_sb[:, qsl], y_ps)
                    linv = npool.tile([1, QTILE], F32, name="linv")
                    nc.vector.reciprocal_approx_fast(linv, l_ps)
                    linv_dr = nbounce.tile([1, QTILE], F32, name="linv_dr")
                    nc.sync.dma_start(linv_dr, linv)
                    bc_sb = npool.tile([128, QTILE], F32, name="bc_sb")
                    nc.sync.dma_start(bc_sb, linv_dr.to_broadcast([128, QTILE]))
                    norm_pairs.append((qsl, bc_sb))

                # normalize at head end: broadcasts already in flight, so these
                # don't block the DVE stream mid-pipeline
                for qsl_n, bc_n in norm_pairs:
                    nc.vector.tensor_tensor(
                        y_sb[:, qsl_n], y_sb[:, qsl_n], bc_n, ALU.mult
                    )

            if h != H_LOC - 1:
                continue
            # output projection for this batch: out^T = Wp.T @ y^T
            for jt in range(NQT):
                tsl = slice(jt * QTILE, (jt + 1) * QTILE)
                for co in range(C // 128):
                    o_ps = opsum.tile([128, QTILE], F32, name="o_ps")
                    for h in range(H_LOC):
                        nc.tensor.matmul(
                            o_ps,
                            wp_sb[:, h, co * 128 : (co + 1) * 128],
                            y_tiles[h][:, tsl],
                            start=(h == 0),
                            stop=(h == H_LOC - 1),
                        )
                    o_sb = opool.tile([128, QTILE], F32, name="o_sb")
                    nc.vector.tensor_copy(o_sb, o_ps)
                    nc.sync.dma_start(
                        out_t[b, co * 128 : (co + 1) * 128, tsl], o_sb
                    )


def _get_nc():
    if "nc" not in _CACHED:
        _CACHED["nc"] = build_nc()
    return _CACHED["nc"]


def kernel(x, sin, cos, W_qkv, W_proj):
    x = np.asarray(x, dtype=np.float32)
    sin = np.asarray(sin, dtype=np.float32)
    cos = np.asarray(cos, dtype=np.float32)
    W_qkv = np.asarray(W_qkv, dtype=np.float32)
    W_proj = np.asarray(W_proj, dtype=np.float32)

    sin_t = np.ascontiguousarray(sin[0, 0].T)  # [HD, T]
    cos_t = np.ascontiguousarray(cos[0, 0].T)
    pt = rope_perm_matrix()
    ones_col = np.ones((128, 1), np.float32)
    ones_row = np.ones((1, 128), np.float32)

    in_maps = []
    for g in range(BGROUPS):
        x_tg = np.ascontiguousarray(
            x[g * B_LOC : (g + 1) * B_LOC].transpose(0, 2, 1)
        )  # [B_LOC, C, T]
        for s in range(HSHARDS):
            qcols = W_qkv[:, s * FQK : (s + 1) * FQK]
            kcols = W_qkv[:, C + s * FQK : C + (s + 1) * FQK]
            vcols = W_qkv[:, 2 * C + s * FV : 2 * C + (s + 1) * FV]
            w_qkv_loc = np.ascontiguousarray(
                np.concatenate([qcols, kcols, vcols], axis=1)
            )
            w_proj_loc = np.ascontiguousarray(W_proj[s * FV : (s + 1) * FV, :])
            in_maps.append(
                {
                    "x_t": x_tg,
                    "w_qkv": w_qkv_loc,
                    "w_proj": w_proj_loc,
                    "sin_t": sin_t,
                    "cos_t": cos_t,
                    "pt": pt,
                    "ones_col": ones_col,
                    "ones_row": ones_row,
                }
            )

    trace = bool(int(os.environ.get("KERNEL_TRACE", "0")))
    if trace:
        _install_ntff_hook()
    nc = _get_nc()
    res = run_bass_kernel_spmd(
        nc, in_maps, core_ids=list(range(NCORES)), trace=trace
    )
    _CACHED["last_result"] = res

    out = np.zeros((B, T, C), dtype=np.float32)
    for g in range(BGROUPS):
        acc = np.zeros((B_LOC, C, T), dtype=np.float32)
        for s in range(HSHARDS):
            acc += res.results[g * HSHARDS + s]["out_t"]
        out[g * B_LOC : (g + 1) * B_LOC] = acc.transpose(0, 2, 1)
    return out



# revision 6
# speedup vs baseline: 1.1176x; 1.1176x over previous
"""Trainium2 Bass kernel for nn_MHA_43095701848407.

MHA forward: qkv = x @ W_qkv, RoPE on q/k, causal softmax attention,
y @ W_proj.  B=4, T=2048, C=2048, 16 heads, head_dim=128.

Sharding (8 cores): tensor-parallel over heads (4 shards x 4 heads) x
data-parallel over batch (2 groups x 2 batches).  core = group*4 + shard.

v2 design (vs v1): all matmul operands bf16 (FWL fast weight loads),
q^T/k^T/v kept SBUF-resident per batch (no DRAM roundtrip), per-batch
pipeline qkv->attn->proj with x prefetched for the next batch, weight
pairing (one lhsT load serves two ap-512 matmuls), v computed transposed
in the W-stream then moved to natural layout via DMA transpose, causal
mask via triangular-mask multiply on DVE, softmax 1/l broadcast via PE
outer product (no DRAM bounce), proj loops reordered for lhsT reuse.
Host sums the 4 head-shard bf16 partials per batch in f32.

Self-contained: shapes/sharding hardcoded; inputs full-size numpy arrays.
"""

import math
import os
import sys
import types

import ml_dtypes
import numpy as np

import concourse.bass as bass
import concourse.mybir as mybir
import concourse.tile as tile
from concourse import bacc
from concourse.bass_utils import run_bass_kernel_spmd

F32 = mybir.dt.float32
BF16 = mybir.dt.bfloat16
AF = mybir.ActivationFunctionType
ALU = mybir.AluOpType
NPBF = ml_dtypes.bfloat16

# Problem shape (hardcoded per contract)
B, T, C = 4, 2048, 2048
H, HD = 16, 128
NCORES = 8
BGROUPS, HSHARDS = 2, 4  # batch groups x head shards
B_LOC = B // BGROUPS  # 2 batches per core
H_LOC = H // HSHARDS  # 4 heads per core
FQK = H_LOC * HD  # 512 features for q (and for k)
FV = H_LOC * HD  # 512 for v
NCH = 12  # qkv feature chunks of 128 (4 q + 4 k + 4 v)
KO = C // 128  # 16 contraction chunks
TSLAB = 1024
NSLAB = T // TSLAB  # 2 t-slabs per batch
QT = 1024  # attention q tile
NQT = T // QT  # 2 q tiles
NKBT = QT // 128  # 8 key blocks per q tile width
SCALE = 1.0 / math.sqrt(HD)

_CACHED = {}


def _install_ntff_hook():
    """Register the axon NTFF profile hook (container's antenv lacks it)."""
    if "antenv.axon_hooks" in sys.modules:
        return
    try:
        mod = types.ModuleType("antenv.axon_hooks")
        holder = [None]
        mod.set_axon_ntff_profile_hook = lambda h: holder.__setitem__(0, h)
        mod.get_axon_ntff_profile_hook = lambda: holder[0]
        sys.modules["antenv.axon_hooks"] = mod
        import antenv

        antenv.axon_hooks = mod
        if "/root/.axon_site" not in sys.path:
            sys.path.insert(0, "/root/.axon_site")
        from trn_agent_boot.trn_boot import _ntff_profile_via_ctypes

        mod.set_axon_ntff_profile_hook(
            _ntff_profile_via_ctypes("/opt/axon/libaxon_pjrt.so")
        )
    except Exception:
        sys.modules.pop("antenv.axon_hooks", None)


def rope_perm_matrix():
    """lhsT for the rotate-half matmul: rot^T = PT.T @ q^T.
    rot[2i] = -q[2i+1], rot[2i+1] = q[2i]."""
    pt = np.zeros((HD, HD), dtype=np.float32)
    for i in range(HD // 2):
        pt[2 * i + 1, 2 * i] = -1.0
        pt[2 * i, 2 * i + 1] = 1.0
    return pt


def build_nc():
    nc = bacc.Bacc("TRN2", target_bir_lowering=False, debug=False)

    x_t = nc.dram_tensor("x_t", [B_LOC, C, T], BF16, kind="ExternalInput").ap()
    w_qkv = nc.dram_tensor("w_qkv", [C, NCH * 128], BF16, kind="ExternalInput").ap()
    w_proj = nc.dram_tensor("w_proj", [FV, C], BF16, kind="ExternalInput").ap()
    cos_t = nc.dram_tensor("cos_t", [HD, T], BF16, kind="ExternalInput").ap()
    sin_t = nc.dram_tensor("sin_t", [HD, T], BF16, kind="ExternalInput").ap()
    pt = nc.dram_tensor("pt", [HD, HD], BF16, kind="ExternalInput").ap()
    ones_col = nc.dram_tensor("ones_col", [128, 1], BF16, kind="ExternalInput").ap()
    ones_row = nc.dram_tensor("ones_row", [1, 128], BF16, kind="ExternalInput").ap()
    tri = nc.dram_tensor("tri", [128, 128], BF16, kind="ExternalInput").ap()
    out_t = nc.dram_tensor("out_t", [B_LOC, C, T], BF16, kind="ExternalOutput").ap()

    with tile.TileContext(nc) as tc:
        with nc.allow_low_precision(reason="bf16 matmuls by design; tol 2e-2"):
            _emit(nc, tc, x_t, w_qkv, w_proj, cos_t, sin_t, pt, ones_col,
                  ones_row, tri, out_t)
    nc.compile()
    return nc


def _emit(nc, tc, x_t, w_qkv, w_proj, cos_t, sin_t, pt, ones_col, ones_row,
          tri, out_t):
    with (
        tc.tile_pool(name="consts", bufs=1) as consts,
        tc.tile_pool(name="wq", bufs=1) as wqpool,
        tc.tile_pool(name="wp", bufs=1) as wppool,
        tc.tile_pool(name="qkres", bufs=1) as qkres,
        tc.tile_pool(name="vres", bufs=1) as vres,
        tc.tile_pool(name="yres", bufs=1) as yres,
        tc.tile_pool(name="xpool", bufs=3) as xpool,
    ):
        pt_sb = consts.tile([HD, HD], BF16)
        nc.sync.dma_start(pt_sb, pt)
        ones_c_sb = consts.tile([128, 1], BF16)
        nc.sync.dma_start(ones_c_sb, ones_col)
        ones_r_sb = consts.tile([1, 128], BF16)
        nc.sync.dma_start(ones_r_sb, ones_row)
        tri_sb = consts.tile([128, 128], BF16)
        nc.sync.dma_start(tri_sb, tri)
        cos_sb = consts.tile([HD, T], BF16)
        nc.sync.dma_start(cos_sb, cos_t)
        sin_sb = consts.tile([HD, T], BF16)
        nc.sync.dma_start(sin_sb, sin_t)

        # W_qkv resident, loaded per f-chunk so chunk 0 is ready fast.
        w_sb = wqpool.tile([128, KO, NCH * 128], BF16)
        w_src = w_qkv.rearrange("(ko p) f -> p ko f", p=128)
        for f in range(NCH):
            fsl = slice(f * 128, (f + 1) * 128)
            nc.scalar.dma_start(w_sb[:, :, fsl], w_src[:, :, fsl])
        wp_sb = wppool.tile([128, H_LOC, C], BF16)
        nc.scalar.dma_start(wp_sb, w_proj.rearrange("(h p) c -> p h c", p=128))

        # Per-batch resident activations (reused across batches; the tile
        # framework serializes WAR hazards between batches automatically).
        qk_sb = qkres.tile([128, 8, T], BF16)  # chunks: q heads 0-3, k heads 4-7
        v_sb = vres.tile([128, T // 128, FV], BF16)  # natural [t, fv]
        y_sb = yres.tile([128, H_LOC, T], BF16)  # y^T per head

        def load_x_half(b, js, hh, split_ko):
            x3 = x_t[b].rearrange("(ko p) t -> p ko t", p=128)
            hsl = slice(js * TSLAB + hh * 512, js * TSLAB + (hh + 1) * 512)
            x_h = xpool.tile([128, KO, 512], BF16, name="x_h")
            if split_ko:
                for ko in range(KO):
                    nc.sync.dma_start(x_h[:, ko, :], x3[:, ko, hsl])
            else:
                nc.sync.dma_start(x_h, x3[:, :, hsl])
            return x_h

        # batch 0: load first three half-slabs up front (4th waits on a slot)
        xq = [load_x_half(0, 0, 0, True), load_x_half(0, 0, 1, True),
              load_x_half(0, 1, 0, False)]

        for b in range(B_LOC):
            halves = {(0, 0): xq[0], (0, 1): xq[1], (1, 0): xq[2]}
            _phase_qkv(nc, tc, b, halves, load_x_half, w_sb, pt_sb, cos_sb,
                       sin_sb, qk_sb, v_sb)
            if b + 1 < B_LOC:
                # prefetch next batch's x during this batch's attention
                xq = [load_x_half(b + 1, 0, 0, False),
                      load_x_half(b + 1, 0, 1, False),
                      load_x_half(b + 1, 1, 0, False)]
            _phase_attn(nc, tc, b, qk_sb, v_sb, y_sb, ones_c_sb, ones_r_sb,
                        tri_sb)
            _phase_proj(nc, tc, b, wp_sb, y_sb, out_t)


def _phase_qkv(nc, tc, b, halves, load_x_half, w_sb, pt_sb, cos_sb, sin_sb,
               qk_sb, v_sb):
    """qkv^T = W.T @ x^T in 128-feature chunks; RoPE on q,k chunks; v
    chunks transposed to natural layout via DMA transpose.

    `halves` holds pre-issued x half-slab tiles for (js, hh) except
    (1, 1), which is loaded here (its SBUF slot frees mid-batch)."""
    with (
        tc.tile_pool(name="rawpool", bufs=2) as rawpool,
        tc.tile_pool(name="rotbf", bufs=2) as rotbfpool,
        tc.tile_pool(name="vtpool", bufs=4) as vtpool,
        tc.tile_pool(name="qkpsA", bufs=2, space="PSUM") as qkpsA,
        tc.tile_pool(name="qkpsB", bufs=2, space="PSUM") as qkpsB,
        tc.tile_pool(name="rotpsA", bufs=2, space="PSUM") as rotpsA,
        tc.tile_pool(name="rotpsB", bufs=2, space="PSUM") as rotpsB,
    ):
        for js in range(NSLAB):
            tsl = slice(js * TSLAB, (js + 1) * TSLAB)
            if js == 1 and (1, 1) not in halves:
                # slot for (1,1) frees once slab-0 compute is done
                halves[(1, 1)] = load_x_half(b, 1, 1, False)
            h0 = halves[(js, 0)]
            h1 = halves[(js, 1)]
            for f in range(NCH):
                fsl = slice(f * 128, (f + 1) * 128)
                psA = qkpsA.tile([128, 512], F32, name="psA")
                psB = qkpsB.tile([128, 512], F32, name="psB")
                # (1,1) arrives late: for the first chunk of slab 1 run
                # the halves unpaired so the PE chews h0 while h1 loads
                paired = not (js == 1 and f == 0)
                if paired:
                    for ko in range(KO):
                        nc.tensor.matmul(psA, w_sb[:, ko, fsl], h0[:, ko, :],
                                         start=(ko == 0), stop=(ko == KO - 1))
                        nc.tensor.matmul(psB, w_sb[:, ko, fsl], h1[:, ko, :],
                                         start=(ko == 0), stop=(ko == KO - 1))
                else:
                    for ko in range(KO):
                        nc.tensor.matmul(psA, w_sb[:, ko, fsl], h0[:, ko, :],
                                         start=(ko == 0), stop=(ko == KO - 1))
                    for ko in range(KO):
                        nc.tensor.matmul(psB, w_sb[:, ko, fsl], h1[:, ko, :],
                                         start=(ko == 0), stop=(ko == KO - 1))
                if f < 8:
                    # q/k chunk: RoPE.  roped = raw*cos + (PT.T@raw)*sin
                    raw = rawpool.tile([128, TSLAB], BF16, name="raw")
                    nc.scalar.copy(raw[:, 0:512], psA)
                    nc.scalar.copy(raw[:, 512:], psB)
                    rpsA = rotpsA.tile([128, 512], F32, name="rpsA")
                    rpsB = rotpsB.tile([128, 512], F32, name="rpsB")
                    nc.tensor.matmul(rpsA, pt_sb, raw[:, 0:512],
                                     start=True, stop=True)
                    nc.tensor.matmul(rpsB, pt_sb, raw[:, 512:],
                                     start=True, stop=True)
                    rot = rotbfpool.tile([128, TSLAB], BF16, name="rot")
                    nc.scalar.copy(rot[:, 0:512], rpsA)
                    nc.scalar.copy(rot[:, 512:], rpsB)
                    # t1 = raw*cos in place (Pool); t2 = rot*sin in place (DVE)
                    nc.gpsimd.tensor_tensor(raw, raw, cos_sb[:, tsl], ALU.mult)
                    nc.vector.tensor_tensor(rot, rot, sin_sb[:, tsl], ALU.mult)
                    nc.vector.tensor_tensor(qk_sb[:, f, tsl], raw, rot, ALU.add)
                else:
                    # v chunk: evacuate v^T then DMA-transpose into v_sb
                    fc = f - 8
                    vt = vtpool.tile([128, TSLAB], BF16, name="vt")
                    nc.scalar.copy(vt[:, 0:512], psA)
                    nc.scalar.copy(vt[:, 512:], psB)
                    for tb in range(TSLAB // 128):
                        nc.sync.dma_start_transpose(
                            v_sb[:, js * (TSLAB // 128) + tb,
                                 fc * 128:(fc + 1) * 128],
                            vt[:, tb * 128:(tb + 1) * 128],
                        )


def _phase_attn(nc, tc, b, qk_sb, v_sb, y_sb, ones_c_sb, ones_r_sb, tri_sb):
    """Causal attention per head, transposed orientation.
    scores^T [k, q] -> exp (ACT) -> tri-mask (DVE) -> l (ones matmul),
    y^T = v_nat.T @ p^T; normalization via PE outer-product broadcast."""
    with (
        tc.tile_pool(name="ppool", bufs=4) as ppool,
        tc.tile_pool(name="nfpool", bufs=2) as nfpool,
        tc.tile_pool(name="nbpool", bufs=2) as nbpool,
        tc.tile_pool(name="bcpool", bufs=4) as bcpool,
        tc.tile_pool(name="spsA", bufs=2, space="PSUM") as spsA,
        tc.tile_pool(name="spsB", bufs=2, space="PSUM") as spsB,
        tc.tile_pool(name="ypsA", bufs=1, space="PSUM") as ypsA,
        tc.tile_pool(name="ypsB", bufs=1, space="PSUM") as ypsB,
        tc.tile_pool(name="lpsA", bufs=1, space="PSUM") as lpsA,
        tc.tile_pool(name="lpsB", bufs=1, space="PSUM") as lpsB,
    ):
        for h in range(H_LOC):
            qt = qk_sb[:, h, :]
            kt = qk_sb[:, 4 + h, :]
            norm_jobs = []
            for jq in range(NQT):
                q0 = jq * QT
                nkb = NKBT * (jq + 1)
                last_a = min(nkb - 1, NKBT * jq + 3)  # last kb touching half A
                y_h = [ypsA.tile([128, 512], F32, name="yA"),
                       ypsB.tile([128, 512], F32, name="yB")]
                l_h = [lpsA.tile([1, 512], F32, name="lA"),
                       lpsB.tile([1, 512], F32, name="lB")]
                # emit with 1-block skew: scores(kb+1) issue while exp(kb)
                # runs on ACT, then l/pv(kb) consume p(kb)
                prev = None
                for kb in range(nkb):
                    s_diag = kb - NKBT * jq
                    qoff = 128 * s_diag if s_diag > 0 else 0
                    ksl = slice(kb * 128, (kb + 1) * 128)
                    boff = max(0, qoff - 512)
                    s_a = None
                    if qoff < 512:
                        s_a = spsA.tile([128, 512], F32, name="sA")
                        nc.tensor.matmul(
                            s_a[:, qoff:], kt[:, ksl],
                            qt[:, q0 + qoff:q0 + 512], start=True, stop=True)
                    s_b = spsB.tile([128, 512], F32, name="sB")
                    nc.tensor.matmul(
                        s_b[:, boff:], kt[:, ksl],
                        qt[:, q0 + 512 + boff:q0 + QT], start=True, stop=True)
                    p_sb = ppool.tile([128, QT], BF16, name="p_sb")
                    if qoff < 512:
                        nc.scalar.activation(p_sb[:, qoff:512], s_a[:, qoff:],
                                             AF.Exp, scale=SCALE)
                    nc.scalar.activation(p_sb[:, 512 + boff:], s_b[:, boff:],
                                         AF.Exp, scale=SCALE)
                    if s_diag >= 0:
                        # causal: zero p where q < k in the diagonal block
                        nc.vector.tensor_tensor(
                            p_sb[:, qoff:qoff + 128], p_sb[:, qoff:qoff + 128],
                            tri_sb, ALU.mult)
                    if prev is not None:
                        _emit_l_pv(nc, v_sb, ones_c_sb, h, l_h, y_h,
                                   last_a, nkb, *prev)
                    prev = (p_sb, kb, qoff, boff)
                _emit_l_pv(nc, v_sb, ones_c_sb, h, l_h, y_h, last_a, nkb,
                           *prev)

                # 1/l then broadcast to [128, 512] via PE outer product
                for hh in range(2):
                    linv = nfpool.tile([1, 512], F32, name="linv")
                    nc.vector.reciprocal_approx_fast(linv, l_h[hh])
                    linv_bf = nbpool.tile([1, 512], BF16, name="linv_bf")
                    nc.vector.tensor_copy(linv_bf, linv)
                    # reuse the scores pools' slots (same tag => same ring)
                    bc_ps = (spsA.tile([128, 512], F32, name="sA") if hh == 0
                             else spsB.tile([128, 512], F32, name="sB"))
                    nc.tensor.matmul(bc_ps, ones_r_sb, linv_bf,
                                     start=True, stop=True)
                    bc_sb = bcpool.tile([128, 512], BF16, name="bc_sb")
                    nc.scalar.copy(bc_sb, bc_ps)
                    # evacuate y unnormalized (frees psum); normalize at
                    # head end once the broadcasts are in flight
                    ysl = slice(q0 + hh * 512, q0 + (hh + 1) * 512)
                    nc.vector.tensor_copy(y_sb[:, h, ysl], y_h[hh])
                    norm_jobs.append((ysl, bc_sb))
            for ysl, bc_sb in norm_jobs:
                nc.vector.tensor_tensor(y_sb[:, h, ysl], y_sb[:, h, ysl],
                                        bc_sb, ALU.mult)


def _emit_l_pv(nc, v_sb, ones_c_sb, h, l_h, y_h, last_a, nkb, p_sb, kb, qoff,
               boff):
    """l += ones.T @ p ; y^T += v_nat.T @ p^T for one key block.
    Half A (q cols [0,512)) ends at last_a; half B at nkb-1."""
    hsl = slice(h * 128, (h + 1) * 128)
    if qoff < 512:
        nc.tensor.matmul(l_h[0][:, qoff:], ones_c_sb, p_sb[:, qoff:512],
                         start=(kb == 0), stop=(kb == last_a))
    nc.tensor.matmul(l_h[1][:, boff:], ones_c_sb, p_sb[:, 512 + boff:],
                     start=(kb == 0), stop=(kb == nkb - 1))
    # one v lhsT load serves both halves
    if qoff < 512:
        nc.tensor.matmul(y_h[0][:, qoff:], v_sb[:, kb, hsl], p_sb[:, qoff:512],
                         start=(kb == 0), stop=(kb == last_a))
    nc.tensor.matmul(y_h[1][:, boff:], v_sb[:, kb, hsl], p_sb[:, 512 + boff:],
                     start=(kb == 0), stop=(kb == nkb - 1))


def _phase_proj(nc, tc, b, wp_sb, y_sb, out_t):
    """out^T[c, t] partial = Wp_loc.T @ y^T.  h-outer/jt-inner so one wp
    lhsT load serves 4 matmuls into 4 psum banks."""
    NJT = T // 512
    with (
        tc.tile_pool(name="opool", bufs=4) as opool,
        tc.tile_pool(name="opsum", bufs=8, space="PSUM") as opsum,
    ):
        for co in range(C // 128):
            csl = slice(co * 128, (co + 1) * 128)
            o_ps = [opsum.tile([128, 512], F32, name="o_ps") for _ in range(NJT)]
            for h in range(H_LOC):
                for jt in range(NJT):
                    nc.tensor.matmul(
                        o_ps[jt], wp_sb[:, h, csl],
                        y_sb[:, h, jt * 512:(jt + 1) * 512],
                        start=(h == 0), stop=(h == H_LOC - 1))
            for jt in range(NJT):
                o_sb = opool.tile([128, 512], BF16, name="o_sb")
                # alternate ACT/DVE for psum evacuation
                if jt % 2 == 0:
                    nc.scalar.copy(o_sb, o_ps[jt])
                else:
                    nc.vector.tensor_copy(o_sb, o_ps[jt])
                nc.sync.dma_start(
                    out_t[b, csl, jt * 512:(jt + 1) * 512], o_sb)


def _get_nc():
    if "nc" not in _CACHED:
        _CACHED["nc"] = build_nc()
    return _CACHED["nc"]


def kernel(x, sin, cos, W_qkv, W_proj):
    x = np.asarray(x, dtype=np.float32)
    sin = np.asarray(sin, dtype=np.float32)
    cos = np.asarray(cos, dtype=np.float32)
    W_qkv = np.asarray(W_qkv, dtype=np.float32)
    W_proj = np.asarray(W_proj, dtype=np.float32)

    sin_t = np.ascontiguousarray(sin[0, 0].T).astype(NPBF)  # [HD, T]
    cos_t = np.ascontiguousarray(cos[0, 0].T).astype(NPBF)
    pt = rope_perm_matrix().astype(NPBF)
    ones_col = np.ones((128, 1), NPBF)
    ones_row = np.ones((1, 128), NPBF)
    tri = np.triu(np.ones((128, 128), np.float32)).astype(NPBF)

    in_maps = []
    for g in range(BGROUPS):
        x_tg = np.ascontiguousarray(
            x[g * B_LOC:(g + 1) * B_LOC].transpose(0, 2, 1)
        ).astype(NPBF)  # [B_LOC, C, T]
        for s in range(HSHARDS):
            qcols = W_qkv[:, s * FQK:(s + 1) * FQK]
            kcols = W_qkv[:, C + s * FQK:C + (s + 1) * FQK]
            vcols = W_qkv[:, 2 * C + s * FV:2 * C + (s + 1) * FV]
            w_qkv_loc = np.ascontiguousarray(
                np.concatenate([qcols, kcols, vcols], axis=1)).astype(NPBF)
            w_proj_loc = np.ascontiguousarray(
                W_proj[s * FV:(s + 1) * FV, :]).astype(NPBF)
            in_maps.append(
                {
                    "x_t": x_tg,
                    "w_qkv": w_qkv_loc,
                    "w_proj": w_proj_loc,
                    "sin_t": sin_t,
                    "cos_t": cos_t,
                    "pt": pt,
                    "ones_col": ones_col,
                    "ones_row": ones_row,
                    "tri": tri,
                }
            )

    trace = bool(int(os.environ.get("KERNEL_TRACE", "0")))
    if trace:
        _install_ntff_hook()
    nc = _get_nc()
    res = run_bass_kernel_spmd(
        nc, in_maps, core_ids=list(range(NCORES)), trace=trace
    )
    _CACHED["last_result"] = res

    out = np.zeros((B, T, C), dtype=np.float32)
    for g in range(BGROUPS):
        acc = np.zeros((B_LOC, C, T), dtype=np.float32)
        for s in range(HSHARDS):
            acc += res.results[g * HSHARDS + s]["out_t"].astype(np.float32)
        out[g * B_LOC:(g + 1) * B_LOC] = acc.transpose(0, 2, 1)
    return out


# revision 8
# speedup vs baseline: 1.1427x; 1.0225x over previous
"""Trainium2 Bass kernel for nn_MHA_43095701848407.

MHA forward: qkv = x @ W_qkv, RoPE on q/k, causal softmax attention,
y @ W_proj.  B=4, T=2048, C=2048, 16 heads, head_dim=128.

Sharding (8 cores): tensor-parallel over heads (4 shards x 4 heads) x
data-parallel over batch (2 groups x 2 batches).  core = group*4 + shard.

v3 design: all matmul operands bf16; q^T/k^T/v SBUF-resident per batch;
per-batch pipeline qkv->attn->proj with x prefetch.  PE wall per matmul
is ~N/2.4GHz + 3ns (weight loads hidden for bf16), so the kernel
minimizes streamed columns and instruction count: RoPE rotate-half via
partition-strided SBUF->SBUF DMA (off the PE), v transposed to natural
layout with one wide DMA-transpose per chunk-slab, 2-bank PSUM tiles
with merged single-instruction exp per key block, causal mask as a
triangular-mask multiply on DVE, softmax 1/l broadcast via PE outer
product, proj loops reordered.  v chunks are computed before q/k so the
v transposes drain during qk compute (no bubble into attention).
Host sums the 4 head-shard bf16 partials per batch in f32.

Self-contained: shapes/sharding hardcoded; inputs full-size numpy arrays.
"""

import math
import os
import sys
import types

import ml_dtypes
import numpy as np

import concourse.bass as bass
import concourse.mybir as mybir
import concourse.tile as tile
from concourse import bacc
from concourse.bass_utils import run_bass_kernel_spmd

F32 = mybir.dt.float32
BF16 = mybir.dt.bfloat16
AF = mybir.ActivationFunctionType
ALU = mybir.AluOpType
NPBF = ml_dtypes.bfloat16

# Problem shape (hardcoded per contract)
B, T, C = 4, 2048, 2048
H, HD = 16, 128
NCORES = 8
BGROUPS, HSHARDS = 2, 4  # batch groups x head shards
B_LOC = B // BGROUPS  # 2 batches per core
H_LOC = H // HSHARDS  # 4 heads per core
FQK = H_LOC * HD  # 512 features for q (and for k)
FV = H_LOC * HD  # 512 for v
NCH = 12  # qkv feature chunks of 128 (4 q + 4 k + 4 v)
# v chunks (8..11) first so their DMA transposes drain during qk compute;
# qk interleaved q0,k0,q1,k1,... so early heads are ready first.
CHUNK_ORDER = [8, 9, 10, 11, 0, 4, 1, 5, 2, 6, 3, 7]
KO = C // 128  # 16 contraction chunks
KOG = 4  # x DMA granularity: 4 ko chunks per transfer
TSLAB = 1024
NSLAB = T // TSLAB  # 2 t-slabs per batch
QT = 1024  # attention q tile
NQT = T // QT  # 2 q tiles
NKBT = QT // 128  # 8 key blocks per q tile width
SCALE = 1.0 / math.sqrt(HD)

_CACHED = {}


def _install_ntff_hook():
    """Register the axon NTFF profile hook (container's antenv lacks it)."""
    if "antenv.axon_hooks" in sys.modules:
        return
    try:
        mod = types.ModuleType("antenv.axon_hooks")
        holder = [None]
        mod.set_axon_ntff_profile_hook = lambda h: holder.__setitem__(0, h)
        mod.get_axon_ntff_profile_hook = lambda: holder[0]
        sys.modules["antenv.axon_hooks"] = mod
        import antenv

        antenv.axon_hooks = mod
        if "/root/.axon_site" not in sys.path:
            sys.path.insert(0, "/root/.axon_site")
        from trn_agent_boot.trn_boot import _ntff_profile_via_ctypes

        mod.set_axon_ntff_profile_hook(
            _ntff_profile_via_ctypes("/opt/axon/libaxon_pjrt.so")
        )
    except Exception:
        sys.modules.pop("antenv.axon_hooks", None)


def build_nc():
    nc = bacc.Bacc("TRN2", target_bir_lowering=False, debug=False)

    x_t = nc.dram_tensor("x_t", [B_LOC, C, T], BF16, kind="ExternalInput").ap()
    w_qkv = nc.dram_tensor("w_qkv", [C, NCH * 128], BF16, kind="ExternalInput").ap()
    w_proj = nc.dram_tensor("w_proj", [FV, C], BF16, kind="ExternalInput").ap()
    cos_t = nc.dram_tensor("cos_t", [HD, T], BF16, kind="ExternalInput").ap()
    sin_t = nc.dram_tensor("sin_t", [HD, T], BF16, kind="ExternalInput").ap()
    ones_col = nc.dram_tensor("ones_col", [128, 1], BF16, kind="ExternalInput").ap()
    ones_row = nc.dram_tensor("ones_row", [1, 128], BF16, kind="ExternalInput").ap()
    tri = nc.dram_tensor("tri", [128, 128], BF16, kind="ExternalInput").ap()
    out_t = nc.dram_tensor("out_t", [B_LOC, C, T], BF16, kind="ExternalOutput").ap()

    with tile.TileContext(nc) as tc:
        with nc.allow_low_precision(reason="bf16 matmuls by design; tol 2e-2"):
            _emit(nc, tc, x_t, w_qkv, w_proj, cos_t, sin_t, ones_col,
                  ones_row, tri, out_t)
    nc.compile()
    return nc


def _emit(nc, tc, x_t, w_qkv, w_proj, cos_t, sin_t, ones_col, ones_row,
          tri, out_t):
    with (
        tc.tile_pool(name="consts", bufs=1) as consts,
        tc.tile_pool(name="wq", bufs=1) as wqpool,
        tc.tile_pool(name="wp", bufs=1) as wppool,
        tc.tile_pool(name="qkres", bufs=1) as qkres,
        tc.tile_pool(name="vres", bufs=1) as vres,
        tc.tile_pool(name="yres", bufs=1) as yres,
        tc.tile_pool(name="xpool", bufs=3) as xpool,
    ):
        ones_c_sb = consts.tile([128, 1], BF16)
        nc.sync.dma_start(ones_c_sb, ones_col)
        ones_r_sb = consts.tile([1, 128], BF16)
        nc.sync.dma_start(ones_r_sb, ones_row)
        tri_sb = consts.tile([128, 128], BF16)
        nc.sync.dma_start(tri_sb, tri)
        cos_sb = consts.tile([HD, T], BF16)
        nc.sync.dma_start(cos_sb, cos_t)
        sin_sb = consts.tile([HD, T], BF16)
        nc.sync.dma_start(sin_sb, sin_t)

        # W_qkv resident, loaded per f-chunk in compute order
        w_sb = wqpool.tile([128, KO, NCH * 128], BF16)
        w_src = w_qkv.rearrange("(ko p) f -> p ko f", p=128)
        for f in CHUNK_ORDER:
            fsl = slice(f * 128, (f + 1) * 128)
            nc.scalar.dma_start(w_sb[:, :, fsl], w_src[:, :, fsl])
        wp_sb = wppool.tile([128, H_LOC, C], BF16)
        nc.scalar.dma_start(wp_sb, w_proj.rearrange("(h p) c -> p h c", p=128))

        # Per-batch resident activations (reused across batches; the tile
        # framework serializes WAR hazards between batches automatically).
        qk_sb = qkres.tile([128, 8, T], BF16)  # chunks: q heads 0-3, k heads 4-7
        v_sb = vres.tile([128, T // 128, FV], BF16)  # natural [t, fv]
        y_sb = yres.tile([128, H_LOC, T], BF16)  # y^T per head

        def load_x_half(b, js, hh):
            x3 = x_t[b].rearrange("(ko p) t -> p ko t", p=128)
            hsl = slice(js * TSLAB + hh * 512, js * TSLAB + (hh + 1) * 512)
            x_h = xpool.tile([128, KO, 512], BF16, name="x_h")
            for kg in range(KO // KOG):
                ks = slice(kg * KOG, (kg + 1) * KOG)
                nc.sync.dma_start(x_h[:, ks, :], x3[:, ks, hsl])
            return x_h

        # batch 0: load first three half-slabs up front (4th waits on a slot)
        xq = [load_x_half(0, 0, 0), load_x_half(0, 0, 1), load_x_half(0, 1, 0)]

        for b in range(B_LOC):
            halves = {(0, 0): xq[0], (0, 1): xq[1], (1, 0): xq[2]}
            _phase_qkv(nc, tc, b, halves, load_x_half, w_sb, cos_sb, sin_sb,
                       qk_sb, v_sb)
            if b + 1 < B_LOC:
                # prefetch next batch's x during this batch's attention
                xq = [load_x_half(b + 1, 0, 0), load_x_half(b + 1, 0, 1),
                      load_x_half(b + 1, 1, 0)]
            _phase_attn(nc, tc, b, qk_sb, v_sb, y_sb, ones_c_sb, ones_r_sb,
                        tri_sb)
            _phase_proj(nc, tc, b, wp_sb, y_sb, out_t)


def _phase_qkv(nc, tc, b, halves, load_x_half, w_sb, cos_sb, sin_sb,
               qk_sb, v_sb):
    """qkv^T = W.T @ x^T in 128-feature chunks (v first, then q/k
    interleaved).  RoPE rotate-half via partition-strided SBUF->SBUF DMA:
    roped = raw*cos + shuf(raw)*sin_signed.  v chunks are evacuated as
    v^T and moved to natural [t, fv] layout with one wide DMA transpose
    per (chunk, slab)."""
    with (
        tc.tile_pool(name="rawpool", bufs=2) as rawpool,
        tc.tile_pool(name="shufpool", bufs=2) as shufpool,
        tc.tile_pool(name="vtpool", bufs=2) as vtpool,
        tc.tile_pool(name="qkps", bufs=3, space="PSUM") as qkps,
    ):
        for js in range(NSLAB):
            tsl = slice(js * TSLAB, (js + 1) * TSLAB)
            if js == 1 and (1, 1) not in halves:
                # slot for (1,1) frees once slab-0 compute is done
                halves[(1, 1)] = load_x_half(b, 1, 1)
            h0 = halves[(js, 0)]
            h1 = halves[(js, 1)]
            for ci, f in enumerate(CHUNK_ORDER):
                fsl = slice(f * 128, (f + 1) * 128)
                ps = qkps.tile([128, TSLAB], F32, name="ps")
                # (1,1) arrives late: for the first chunk of slab 1 run
                # the halves unpaired so the PE chews h0 while h1 loads
                if js == 1 and ci == 0:
                    for ko in range(KO):
                        nc.tensor.matmul(ps[:, 0:512], w_sb[:, ko, fsl],
                                         h0[:, ko, :],
                                         start=(ko == 0), stop=(ko == KO - 1))
                    for ko in range(KO):
                        nc.tensor.matmul(ps[:, 512:], w_sb[:, ko, fsl],
                                         h1[:, ko, :],
                                         start=(ko == 0), stop=(ko == KO - 1))
                else:
                    for ko in range(KO):
                        nc.tensor.matmul(ps[:, 0:512], w_sb[:, ko, fsl],
                                         h0[:, ko, :],
                                         start=(ko == 0), stop=(ko == KO - 1))
                        nc.tensor.matmul(ps[:, 512:], w_sb[:, ko, fsl],
                                         h1[:, ko, :],
                                         start=(ko == 0), stop=(ko == KO - 1))
                if f < 8:
                    # q/k chunk: RoPE
                    raw = rawpool.tile([128, TSLAB], BF16, name="raw")
                    nc.scalar.copy(raw, ps)
                    shuf = shufpool.tile([128, TSLAB], BF16, name="shuf")
                    # rotate-half pair swap across adjacent partitions;
                    # issued on the scalar queue right after the evac
                    nc.scalar.dma_start(shuf[0:127:2, :], raw[1:128:2, :])
                    nc.scalar.dma_start(shuf[1:128:2, :], raw[0:127:2, :])
                    # t1 = raw*cos in place (Pool); t2 = shuf*sin_signed
                    # in place (DVE); sum into the resident qk chunk
                    nc.gpsimd.tensor_tensor(raw, raw, cos_sb[:, tsl], ALU.mult)
                    nc.vector.tensor_tensor(shuf, shuf, sin_sb[:, tsl],
                                            ALU.mult)
                    nc.vector.tensor_tensor(qk_sb[:, f, tsl], raw, shuf,
                                            ALU.add)
                else:
                    # v chunk: evacuate v^T, wide-transpose into v_sb
                    fc = f - 8
                    vt = vtpool.tile([128, TSLAB], BF16, name="vt")
                    nc.scalar.copy(vt, ps)
                    nc.sync.dma_start_transpose(
                        v_sb[:, js * (TSLAB // 128):(js + 1) * (TSLAB // 128),
                             fc * 128:(fc + 1) * 128],
                        vt)


def _phase_attn(nc, tc, b, qk_sb, v_sb, y_sb, ones_c_sb, ones_r_sb, tri_sb):
    """Causal attention per head, transposed orientation.
    scores^T [k, q] -> exp (single merged ACT instr) -> tri-mask (DVE) ->
    l (ones matmul), y^T = v_nat.T @ p^T; normalization via PE
    outer-product broadcast."""
    with (
        tc.tile_pool(name="ppool", bufs=4) as ppool,
        tc.tile_pool(name="nfpool", bufs=1) as nfpool,
        tc.tile_pool(name="nbpool", bufs=1) as nbpool,
        tc.tile_pool(name="bcpool", bufs=2) as bcpool,
        tc.tile_pool(name="sps", bufs=2, space="PSUM") as sps,
        tc.tile_pool(name="yps", bufs=1, space="PSUM") as yps,
        tc.tile_pool(name="lps", bufs=1, space="PSUM") as lps,
    ):
        for h in range(H_LOC):
            qt = qk_sb[:, h, :]
            kt = qk_sb[:, 4 + h, :]
            for jq in range(NQT):
                q0 = jq * QT
                nkb = NKBT * (jq + 1)
                last_a = min(nkb - 1, NKBT * jq + 3)  # last kb touching half A
                y_ps = yps.tile([128, QT], F32, name="y_ps")
                l_ps = lps.tile([1, QT], F32, name="l_ps")
                # 1-block skew: scores(kb+1) issue while exp(kb) runs on
                # ACT, then l/pv(kb) consume p(kb)
                prev = None
                for kb in range(nkb):
                    s_diag = kb - NKBT * jq
                    qoff = 128 * s_diag if s_diag > 0 else 0
                    ksl = slice(kb * 128, (kb + 1) * 128)
                    boff = max(0, qoff - 512)
                    s_ps = sps.tile([128, QT], F32, name="s_ps")
                    if qoff < 512:
                        nc.tensor.matmul(
                            s_ps[:, qoff:512], kt[:, ksl],
                            qt[:, q0 + qoff:q0 + 512], start=True, stop=True)
                    nc.tensor.matmul(
                        s_ps[:, 512 + boff:], kt[:, ksl],
                        qt[:, q0 + 512 + boff:q0 + QT], start=True, stop=True)
                    p_sb = ppool.tile([128, QT], BF16, name="p_sb")
                    nc.scalar.activation(p_sb[:, qoff:], s_ps[:, qoff:],
                                         AF.Exp, scale=SCALE)
                    if s_diag >= 0:
                        # causal: zero p where q < k in the diagonal block
                        nc.vector.tensor_tensor(
                            p_sb[:, qoff:qoff + 128], p_sb[:, qoff:qoff + 128],
                            tri_sb, ALU.mult)
                    if prev is not None:
                        _emit_l_pv(nc, v_sb, ones_c_sb, h, l_ps, y_ps,
                                   last_a, nkb, *prev)
                    prev = (p_sb, kb, qoff, boff)
                _emit_l_pv(nc, v_sb, ones_c_sb, h, l_ps, y_ps, last_a, nkb,
                           *prev)

                # 1/l then broadcast to [128, 512] halves via PE outer
                # product
                linv = nfpool.tile([1, QT], F32, name="linv")
                nc.vector.reciprocal_approx_fast(linv, l_ps)
                linv_bf = nbpool.tile([1, QT], BF16, name="linv_bf")
                nc.vector.tensor_copy(linv_bf, linv)
                # y evacuated unnormalized in one cross-bank copy (frees
                # psum); normalized at head end once broadcasts land
                nc.vector.tensor_copy(y_sb[:, h, q0:q0 + QT], y_ps)
                for hh in range(2):
                    bc_ps = sps.tile([128, QT], F32, name="s_ps")
                    nc.tensor.matmul(bc_ps[:, 0:512], ones_r_sb,
                                     linv_bf[:, hh * 512:(hh + 1) * 512],
                                     start=True, stop=True)
                    bc_sb = bcpool.tile([128, 512], BF16, name="bc_sb")
                    nc.scalar.copy(bc_sb, bc_ps[:, 0:512])
                    ysl = slice(q0 + hh * 512, q0 + (hh + 1) * 512)
                    nc.vector.tensor_tensor(y_sb[:, h, ysl], y_sb[:, h, ysl],
                                            bc_sb, ALU.mult)


def _emit_l_pv(nc, v_sb, ones_c_sb, h, l_ps, y_ps, last_a, nkb, p_sb, kb,
               qoff, boff):
    """l += ones.T @ p ; y^T += v_nat.T @ p^T for one key block.
    Half A (q cols [0,512)) ends at last_a; half B at nkb-1."""
    hsl = slice(h * 128, (h + 1) * 128)
    if qoff < 512:
        nc.tensor.matmul(l_ps[:, qoff:512], ones_c_sb, p_sb[:, qoff:512],
                         start=(kb == 0), stop=(kb == last_a))
    nc.tensor.matmul(l_ps[:, 512 + boff:], ones_c_sb, p_sb[:, 512 + boff:],
                     start=(kb == 0), stop=(kb == nkb - 1))
    # one v lhsT load serves both halves
    if qoff < 512:
        nc.tensor.matmul(y_ps[:, qoff:512], v_sb[:, kb, hsl],
                         p_sb[:, qoff:512],
                         start=(kb == 0), stop=(kb == last_a))
    nc.tensor.matmul(y_ps[:, 512 + boff:], v_sb[:, kb, hsl],
                     p_sb[:, 512 + boff:],
                     start=(kb == 0), stop=(kb == nkb - 1))


def _phase_proj(nc, tc, b, wp_sb, y_sb, out_t):
    """out^T[c, t] partial = Wp_loc.T @ y^T.  h-outer/jt-inner so one wp
    lhsT load serves 4 matmuls into 4 psum banks."""
    NJT = T // 512
    with (
        tc.tile_pool(name="opool", bufs=4) as opool,
        tc.tile_pool(name="opsum", bufs=8, space="PSUM") as opsum,
    ):
        for co in range(C // 128):
            csl = slice(co * 128, (co + 1) * 128)
            o_ps = [opsum.tile([128, 512], F32, name="o_ps") for _ in range(NJT)]
            for h in range(H_LOC):
                for jt in range(NJT):
                    nc.tensor.matmul(
                        o_ps[jt], wp_sb[:, h, csl],
                        y_sb[:, h, jt * 512:(jt + 1) * 512],
                        start=(h == 0), stop=(h == H_LOC - 1))
            for jt in range(NJT):
                o_sb = opool.tile([128, 512], BF16, name="o_sb")
                # alternate ACT/DVE for psum evacuation
                if jt % 2 == 0:
                    nc.scalar.copy(o_sb, o_ps[jt])
                else:
                    nc.vector.tensor_copy(o_sb, o_ps[jt])
                nc.sync.dma_start(
                    out_t[b, csl, jt * 512:(jt + 1) * 512], o_sb)


def _get_nc():
    if "nc" not in _CACHED:
        _CACHED["nc"] = build_nc()
    return _CACHED["nc"]


def kernel(x, sin, cos, W_qkv, W_proj):
    x = np.asarray(x, dtype=np.float32)
    sin = np.asarray(sin, dtype=np.float32)
    cos = np.asarray(cos, dtype=np.float32)
    W_qkv = np.asarray(W_qkv, dtype=np.float32)
    W_proj = np.asarray(W_proj, dtype=np.float32)

    # rotate-half is a pure pair swap on chip; the sign lives in sin:
    # roped[2i] = raw[2i]cos - raw[2i+1]sin ; roped[2i+1] = raw[2i+1]cos
    # + raw[2i]sin  =>  sin row 2i negated.
    sin_tn = np.ascontiguousarray(sin[0, 0].T).copy()  # [HD, T]
    sin_tn[0::2, :] *= -1.0
    sin_t = sin_tn.astype(NPBF)
    cos_t = np.ascontiguousarray(cos[0, 0].T).astype(NPBF)
    ones_col = np.ones((128, 1), NPBF)
    ones_row = np.ones((1, 128), NPBF)
    tri = np.triu(np.ones((128, 128), np.float32)).astype(NPBF)

    in_maps = []
    for g in range(BGROUPS):
        x_tg = np.ascontiguousarray(
            x[g * B_LOC:(g + 1) * B_LOC].transpose(0, 2, 1)
        ).astype(NPBF)  # [B_LOC, C, T]
        for s in range(HSHARDS):
            qcols = W_qkv[:, s * FQK:(s + 1) * FQK]
            kcols = W_qkv[:, C + s * FQK:C + (s + 1) * FQK]
            vcols = W_qkv[:, 2 * C + s * FV:2 * C + (s + 1) * FV]
            w_qkv_loc = np.ascontiguousarray(
                np.concatenate([qcols, kcols, vcols], axis=1)).astype(NPBF)
            w_proj_loc = np.ascontiguousarray(
                W_proj[s * FV:(s + 1) * FV, :]).astype(NPBF)
            in_maps.append(
                {
                    "x_t": x_tg,
                    "w_qkv": w_qkv_loc,
                    "w_proj": w_proj_loc,
                    "sin_t": sin_t,
                    "cos_t": cos_t,
                    "ones_col": ones_col,
                    "ones_row": ones_row,
                    "tri": tri,
                }
            )

    trace = bool(int(os.environ.get("KERNEL_TRACE", "0")))
    if trace:
        _install_ntff_hook()
    nc = _get_nc()
    res = run_bass_kernel_spmd(
        nc, in_maps, core_ids=list(range(NCORES)), trace=trace
    )
    _CACHED["last_result"] = res

    out = np.zeros((B, T, C), dtype=np.float32)
    for g in range(BGROUPS):
        acc = np.zeros((B_LOC, C, T), dtype=np.float32)
        for s in range(HSHARDS):
            acc += res.results[g * HSHARDS + s]["out_t"].astype(np.float32)
        out[g * B_LOC:(g + 1) * B_LOC] = acc.transpose(0, 2, 1)
    return out


# revision 9
# speedup vs baseline: 1.2714x; 1.1127x over previous
"""Trainium2 Bass kernel for nn_MHA_43095701848407.

MHA forward: qkv = x @ W_qkv, RoPE on q/k, causal softmax attention,
y @ W_proj.  B=4, T=2048, C=2048, 16 heads, head_dim=128.

Sharding (8 cores): tensor-parallel over heads (4 shards x 4 heads) x
data-parallel over batch (2 groups x 2 batches).  core = group*4 + shard.

v3 design: all matmul operands bf16; q^T/k^T/v SBUF-resident per batch;
per-batch pipeline qkv->attn->proj with x prefetch.  PE wall per matmul
is ~N/2.4GHz + 3ns (weight loads hidden for bf16), so the kernel
minimizes streamed columns and instruction count: RoPE rotate-half via
partition-strided SBUF->SBUF DMA (off the PE), v transposed to natural
layout with one wide DMA-transpose per chunk-slab, 2-bank PSUM tiles
with merged single-instruction exp per key block, causal mask as a
triangular-mask multiply on DVE, softmax 1/l broadcast via PE outer
product, proj loops reordered.  v chunks are computed before q/k so the
v transposes drain during qk compute (no bubble into attention).
Host sums the 4 head-shard bf16 partials per batch in f32.

Self-contained: shapes/sharding hardcoded; inputs full-size numpy arrays.
"""

import math
import os
import sys
import types

import ml_dtypes
import numpy as np

import concourse.bass as bass
import concourse.mybir as mybir
import concourse.tile as tile
from concourse import bacc
from concourse.bass_utils import run_bass_kernel_spmd

F32 = mybir.dt.float32
BF16 = mybir.dt.bfloat16
AF = mybir.ActivationFunctionType
ALU = mybir.AluOpType
NPBF = ml_dtypes.bfloat16

# Problem shape (hardcoded per contract)
B, T, C = 4, 2048, 2048
H, HD = 16, 128
NCORES = 8
BGROUPS, HSHARDS = 2, 4  # batch groups x head shards
B_LOC = B // BGROUPS  # 2 batches per core
H_LOC = H // HSHARDS  # 4 heads per core
FQK = H_LOC * HD  # 512 features for q (and for k)
FV = H_LOC * HD  # 512 for v
NCH = 12  # qkv feature chunks of 128 (4 q + 4 k + 4 v)
# v chunks (8..11) first so their DMA transposes drain during qk compute;
# qk interleaved q0,k0,q1,k1,... so early heads are ready first.
CHUNK_ORDER = [8, 9, 10, 11, 0, 4, 1, 5, 2, 6, 3, 7]
KO = C // 128  # 16 contraction chunks
KOG = 4  # x DMA granularity: 4 ko chunks per transfer
TSLAB = 1024
NSLAB = T // TSLAB  # 2 t-slabs per batch
QT = 1024  # attention q tile
NQT = T // QT  # 2 q tiles
NKBT = QT // 128  # 8 key blocks per q tile width
SCALE = 1.0 / math.sqrt(HD)

_CACHED = {}


def _install_ntff_hook():
    """Register the axon NTFF profile hook (container's antenv lacks it)."""
    if "antenv.axon_hooks" in sys.modules:
        return
    try:
        mod = types.ModuleType("antenv.axon_hooks")
        holder = [None]
        mod.set_axon_ntff_profile_hook = lambda h: holder.__setitem__(0, h)
        mod.get_axon_ntff_profile_hook = lambda: holder[0]
        sys.modules["antenv.axon_hooks"] = mod
        import antenv

        antenv.axon_hooks = mod
        if "/root/.axon_site" not in sys.path:
            sys.path.insert(0, "/root/.axon_site")
        from trn_agent_boot.trn_boot import _ntff_profile_via_ctypes

        mod.set_axon_ntff_profile_hook(
            _ntff_profile_via_ctypes("/opt/axon/libaxon_pjrt.so")
        )
    except Exception:
        sys.modules.pop("antenv.axon_hooks", None)


def build_nc():
    nc = bacc.Bacc("TRN2", target_bir_lowering=False, debug=False)

    x_t = nc.dram_tensor("x_t", [B_LOC, C, T], BF16, kind="ExternalInput").ap()
    w_qkv = nc.dram_tensor("w_qkv", [128, NCH, KO, 128], BF16,
                           kind="ExternalInput").ap()
    w_proj = nc.dram_tensor("w_proj", [FV, C], BF16, kind="ExternalInput").ap()
    cos_t = nc.dram_tensor("cos_t", [HD, T], BF16, kind="ExternalInput").ap()
    sin_t = nc.dram_tensor("sin_t", [HD, T], BF16, kind="ExternalInput").ap()
    ones_col = nc.dram_tensor("ones_col", [128, 1], BF16, kind="ExternalInput").ap()
    ones_row = nc.dram_tensor("ones_row", [1, 128], BF16, kind="ExternalInput").ap()
    tri = nc.dram_tensor("tri", [128, 128], BF16, kind="ExternalInput").ap()
    out_t = nc.dram_tensor("out_t", [B_LOC, C, T], BF16, kind="ExternalOutput").ap()

    with tile.TileContext(nc) as tc:
        with nc.allow_low_precision(reason="bf16 matmuls by design; tol 2e-2"):
            _emit(nc, tc, x_t, w_qkv, w_proj, cos_t, sin_t, ones_col,
                  ones_row, tri, out_t)
    nc.compile()
    return nc


def _emit(nc, tc, x_t, w_qkv, w_proj, cos_t, sin_t, ones_col, ones_row,
          tri, out_t):
    with (
        tc.tile_pool(name="consts", bufs=1) as consts,
        tc.tile_pool(name="wq", bufs=1) as wqpool,
        tc.tile_pool(name="wp", bufs=1) as wppool,
        tc.tile_pool(name="qkres", bufs=1) as qkres,
        tc.tile_pool(name="vres", bufs=1) as vres,
        tc.tile_pool(name="yres", bufs=1) as yres,
        tc.tile_pool(name="xpool", bufs=3) as xpool,
    ):
        ones_c_sb = consts.tile([128, 1], BF16)
        nc.sync.dma_start(ones_c_sb, ones_col)
        ones_r_sb = consts.tile([1, 128], BF16)
        nc.sync.dma_start(ones_r_sb, ones_row)
        tri_sb = consts.tile([128, 128], BF16)
        nc.sync.dma_start(tri_sb, tri)
        cos_sb = consts.tile([HD, T], BF16)
        nc.sync.dma_start(cos_sb, cos_t)
        sin_sb = consts.tile([HD, T], BF16)
        nc.sync.dma_start(sin_sb, sin_t)

        # W_qkv resident, loaded per f-chunk in compute order
        # (host layout [p, chunk, ko, f] gives 4KB-contiguous runs)
        w_sb = wqpool.tile([128, NCH, KO, 128], BF16)
        for f in CHUNK_ORDER:
            nc.scalar.dma_start(w_sb[:, f, :, :], w_qkv[:, f, :, :])
        wp_sb = wppool.tile([128, H_LOC, C], BF16)
        nc.scalar.dma_start(wp_sb, w_proj.rearrange("(h p) c -> p h c", p=128))

        # Per-batch resident activations (reused across batches; the tile
        # framework serializes WAR hazards between batches automatically).
        qk_sb = qkres.tile([128, 8, T], BF16)  # chunks: q heads 0-3, k heads 4-7
        v_sb = vres.tile([128, T // 128, FV], BF16)  # natural [t, fv]
        y_sb = yres.tile([128, H_LOC, T], BF16)  # y^T per head

        def load_x_half(b, js, hh):
            x3 = x_t[b].rearrange("(ko p) t -> p ko t", p=128)
            hsl = slice(js * TSLAB + hh * 512, js * TSLAB + (hh + 1) * 512)
            x_h = xpool.tile([128, KO, 512], BF16, name="x_h")
            for kg in range(KO // KOG):
                ks = slice(kg * KOG, (kg + 1) * KOG)
                nc.sync.dma_start(x_h[:, ks, :], x3[:, ks, hsl])
            return x_h

        # batch 0: load first three half-slabs up front (4th waits on a slot)
        xq = [load_x_half(0, 0, 0), load_x_half(0, 0, 1), load_x_half(0, 1, 0)]

        for b in range(B_LOC):
            halves = {(0, 0): xq[0], (0, 1): xq[1], (1, 0): xq[2]}
            _phase_qkv(nc, tc, b, halves, load_x_half, w_sb, cos_sb, sin_sb,
                       qk_sb, v_sb)
            if b + 1 < B_LOC:
                # prefetch next batch's x during this batch's attention
                xq = [load_x_half(b + 1, 0, 0), load_x_half(b + 1, 0, 1),
                      load_x_half(b + 1, 1, 0)]
            _phase_attn(nc, tc, b, qk_sb, v_sb, y_sb, ones_c_sb, ones_r_sb,
                        tri_sb)
            _phase_proj(nc, tc, b, wp_sb, y_sb, out_t)


def _phase_qkv(nc, tc, b, halves, load_x_half, w_sb, cos_sb, sin_sb,
               qk_sb, v_sb):
    """qkv^T = W.T @ x^T in 128-feature chunks (v first, then q/k
    interleaved).  RoPE rotate-half via partition-strided SBUF->SBUF DMA:
    roped = raw*cos + shuf(raw)*sin_signed.  v chunks are evacuated as
    v^T and moved to natural [t, fv] layout with one wide DMA transpose
    per (chunk, slab)."""
    with (
        tc.tile_pool(name="rawpool", bufs=2) as rawpool,
        tc.tile_pool(name="shufpool", bufs=2) as shufpool,
        tc.tile_pool(name="vtpool", bufs=2) as vtpool,
        tc.tile_pool(name="qkps", bufs=3, space="PSUM") as qkps,
    ):
        for js in range(NSLAB):
            tsl = slice(js * TSLAB, (js + 1) * TSLAB)
            if js == 1 and (1, 1) not in halves:
                # slot for (1,1) frees once slab-0 compute is done
                halves[(1, 1)] = load_x_half(b, 1, 1)
            h0 = halves[(js, 0)]
            h1 = halves[(js, 1)]
            for ci, f in enumerate(CHUNK_ORDER):
                ps = qkps.tile([128, TSLAB], F32, name="ps")
                # (1,1) arrives late: for the first chunk of slab 1 run
                # the halves unpaired so the PE chews h0 while h1 loads
                if js == 1 and ci == 0:
                    for ko in range(KO):
                        nc.tensor.matmul(ps[:, 0:512], w_sb[:, f, ko, :],
                                         h0[:, ko, :],
                                         start=(ko == 0), stop=(ko == KO - 1))
                    for ko in range(KO):
                        nc.tensor.matmul(ps[:, 512:], w_sb[:, f, ko, :],
                                         h1[:, ko, :],
                                         start=(ko == 0), stop=(ko == KO - 1))
                else:
                    for ko in range(KO):
                        nc.tensor.matmul(ps[:, 0:512], w_sb[:, f, ko, :],
                                         h0[:, ko, :],
                                         start=(ko == 0), stop=(ko == KO - 1))
                        nc.tensor.matmul(ps[:, 512:], w_sb[:, f, ko, :],
                                         h1[:, ko, :],
                                         start=(ko == 0), stop=(ko == KO - 1))
                if f < 8:
                    # q/k chunk: RoPE
                    raw = rawpool.tile([128, TSLAB], BF16, name="raw")
                    nc.scalar.copy(raw, ps)
                    shuf = shufpool.tile([128, TSLAB], BF16, name="shuf")
                    # rotate-half pair swap across adjacent partitions;
                    # issued on the scalar queue right after the evac
                    nc.scalar.dma_start(shuf[0:127:2, :], raw[1:128:2, :])
                    nc.scalar.dma_start(shuf[1:128:2, :], raw[0:127:2, :])
                    # t1 = raw*cos in place (Pool); t2 = shuf*sin_signed
                    # in place (DVE); sum into the resident qk chunk
                    nc.gpsimd.tensor_tensor(raw, raw, cos_sb[:, tsl], ALU.mult)
                    nc.vector.tensor_tensor(shuf, shuf, sin_sb[:, tsl],
                                            ALU.mult)
                    nc.vector.tensor_tensor(qk_sb[:, f, tsl], raw, shuf,
                                            ALU.add)
                else:
                    # v chunk: evacuate v^T, wide-transpose into v_sb
                    fc = f - 8
                    vt = vtpool.tile([128, TSLAB], BF16, name="vt")
                    nc.scalar.copy(vt, ps)
                    nc.sync.dma_start_transpose(
                        v_sb[:, js * (TSLAB // 128):(js + 1) * (TSLAB // 128),
                             fc * 128:(fc + 1) * 128],
                        vt)


def _phase_attn(nc, tc, b, qk_sb, v_sb, y_sb, ones_c_sb, ones_r_sb, tri_sb):
    """Causal attention per head, transposed orientation.
    scores^T [k, q] -> exp (single merged ACT instr) -> tri-mask (DVE) ->
    l (ones matmul), y^T = v_nat.T @ p^T; normalization via PE
    outer-product broadcast."""
    with (
        tc.tile_pool(name="ppool", bufs=4) as ppool,
        tc.tile_pool(name="nfpool", bufs=1) as nfpool,
        tc.tile_pool(name="nbpool", bufs=1) as nbpool,
        tc.tile_pool(name="bcpool", bufs=2) as bcpool,
        tc.tile_pool(name="sps", bufs=2, space="PSUM") as sps,
        tc.tile_pool(name="yps", bufs=1, space="PSUM") as yps,
        tc.tile_pool(name="lps", bufs=1, space="PSUM") as lps,
    ):
        for h in range(H_LOC):
            qt = qk_sb[:, h, :]
            kt = qk_sb[:, 4 + h, :]
            for jq in range(NQT):
                q0 = jq * QT
                nkb = NKBT * (jq + 1)
                last_a = min(nkb - 1, NKBT * jq + 3)  # last kb touching half A
                y_ps = yps.tile([128, QT], F32, name="y_ps")
                l_ps = lps.tile([1, QT], F32, name="l_ps")
                # 1-block skew: scores(kb+1) issue while exp(kb) runs on
                # ACT, then l/pv(kb) consume p(kb)
                prev = None
                for kb in range(nkb):
                    s_diag = kb - NKBT * jq
                    qoff = 128 * s_diag if s_diag > 0 else 0
                    ksl = slice(kb * 128, (kb + 1) * 128)
                    boff = max(0, qoff - 512)
                    s_ps = sps.tile([128, QT], F32, name="s_ps")
                    if qoff < 512:
                        nc.tensor.matmul(
                            s_ps[:, qoff:512], kt[:, ksl],
                            qt[:, q0 + qoff:q0 + 512], start=True, stop=True)
                    nc.tensor.matmul(
                        s_ps[:, 512 + boff:], kt[:, ksl],
                        qt[:, q0 + 512 + boff:q0 + QT], start=True, stop=True)
                    p_sb = ppool.tile([128, QT], BF16, name="p_sb")
                    nc.scalar.activation(p_sb[:, qoff:], s_ps[:, qoff:],
                                         AF.Exp, scale=SCALE)
                    if s_diag >= 0:
                        # causal: zero p where q < k in the diagonal block
                        nc.vector.tensor_tensor(
                            p_sb[:, qoff:qoff + 128], p_sb[:, qoff:qoff + 128],
                            tri_sb, ALU.mult)
                    if prev is not None:
                        _emit_l_pv(nc, v_sb, ones_c_sb, h, l_ps, y_ps,
                                   last_a, nkb, *prev)
                    prev = (p_sb, kb, qoff, boff)
                _emit_l_pv(nc, v_sb, ones_c_sb, h, l_ps, y_ps, last_a, nkb,
                           *prev)

                # 1/l, broadcast partition 0 -> 128 on the idle Pool
                # engine (keeps the PE stream free of the softmax tail)
                linv = nfpool.tile([1, QT], F32, name="linv")
                nc.vector.reciprocal_approx_fast(linv, l_ps)
                linv_bf = nbpool.tile([1, QT], BF16, name="linv_bf")
                nc.vector.tensor_copy(linv_bf, linv)
                # y evacuated unnormalized in one cross-bank copy (frees
                # psum); normalized once broadcasts land
                nc.vector.tensor_copy(y_sb[:, h, q0:q0 + QT], y_ps)
                for hh in range(2):
                    bc_sb = bcpool.tile([128, 512], BF16, name="bc_sb")
                    nc.gpsimd.partition_broadcast(
                        bc_sb, linv_bf[:, hh * 512:(hh + 1) * 512])
                    ysl = slice(q0 + hh * 512, q0 + (hh + 1) * 512)
                    nc.vector.tensor_tensor(y_sb[:, h, ysl], y_sb[:, h, ysl],
                                            bc_sb, ALU.mult)


def _emit_l_pv(nc, v_sb, ones_c_sb, h, l_ps, y_ps, last_a, nkb, p_sb, kb,
               qoff, boff):
    """l += ones.T @ p ; y^T += v_nat.T @ p^T for one key block.
    Half A (q cols [0,512)) ends at last_a; half B at nkb-1."""
    hsl = slice(h * 128, (h + 1) * 128)
    if qoff < 512:
        nc.tensor.matmul(l_ps[:, qoff:512], ones_c_sb, p_sb[:, qoff:512],
                         start=(kb == 0), stop=(kb == last_a))
    nc.tensor.matmul(l_ps[:, 512 + boff:], ones_c_sb, p_sb[:, 512 + boff:],
                     start=(kb == 0), stop=(kb == nkb - 1))
    # one v lhsT load serves both halves
    if qoff < 512:
        nc.tensor.matmul(y_ps[:, qoff:512], v_sb[:, kb, hsl],
                         p_sb[:, qoff:512],
                         start=(kb == 0), stop=(kb == last_a))
    nc.tensor.matmul(y_ps[:, 512 + boff:], v_sb[:, kb, hsl],
                     p_sb[:, 512 + boff:],
                     start=(kb == 0), stop=(kb == nkb - 1))


def _phase_proj(nc, tc, b, wp_sb, y_sb, out_t):
    """out^T[c, t] partial = Wp_loc.T @ y^T.  h-outer/jt-inner so one wp
    lhsT load serves 4 matmuls into 4 psum banks."""
    NJT = T // 512
    with (
        tc.tile_pool(name="opool", bufs=4) as opool,
        tc.tile_pool(name="opsum", bufs=8, space="PSUM") as opsum,
    ):
        for co in range(C // 128):
            csl = slice(co * 128, (co + 1) * 128)
            o_ps = [opsum.tile([128, 512], F32, name="o_ps") for _ in range(NJT)]
            for h in range(H_LOC):
                for jt in range(NJT):
                    nc.tensor.matmul(
                        o_ps[jt], wp_sb[:, h, csl],
                        y_sb[:, h, jt * 512:(jt + 1) * 512],
                        start=(h == 0), stop=(h == H_LOC - 1))
            for jt in range(NJT):
                o_sb = opool.tile([128, 512], BF16, name="o_sb")
                # alternate ACT/DVE for psum evacuation
                if jt % 2 == 0:
                    nc.scalar.copy(o_sb, o_ps[jt])
                else:
                    nc.vector.tensor_copy(o_sb, o_ps[jt])
                nc.sync.dma_start(
                    out_t[b, csl, jt * 512:(jt + 1) * 512], o_sb)


def _get_nc():
    if "nc" not in _CACHED:
        _CACHED["nc"] = build_nc()
    return _CACHED["nc"]


def kernel(x, sin, cos, W_qkv, W_proj):
    x = np.asarray(x, dtype=np.float32)
    sin = np.asarray(sin, dtype=np.float32)
    cos = np.asarray(cos, dtype=np.float32)
    W_qkv = np.asarray(W_qkv, dtype=np.float32)
    W_proj = np.asarray(W_proj, dtype=np.float32)

    # rotate-half is a pure pair swap on chip; the sign lives in sin:
    # roped[2i] = raw[2i]cos - raw[2i+1]sin ; roped[2i+1] = raw[2i+1]cos
    # + raw[2i]sin  =>  sin row 2i negated.
    sin_tn = np.ascontiguousarray(sin[0, 0].T).copy()  # [HD, T]
    sin_tn[0::2, :] *= -1.0
    sin_t = sin_tn.astype(NPBF)
    cos_t = np.ascontiguousarray(cos[0, 0].T).astype(NPBF)
    ones_col = np.ones((128, 1), NPBF)
    ones_row = np.ones((1, 128), NPBF)
    tri = np.triu(np.ones((128, 128), np.float32)).astype(NPBF)

    in_maps = []
    for g in range(BGROUPS):
        x_tg = np.ascontiguousarray(
            x[g * B_LOC:(g + 1) * B_LOC].transpose(0, 2, 1)
        ).astype(NPBF)  # [B_LOC, C, T]
        for s in range(HSHARDS):
            qcols = W_qkv[:, s * FQK:(s + 1) * FQK]
            kcols = W_qkv[:, C + s * FQK:C + (s + 1) * FQK]
            vcols = W_qkv[:, 2 * C + s * FV:2 * C + (s + 1) * FV]
            w_flat = np.concatenate([qcols, kcols, vcols], axis=1)
            # [C, 1536] -> [p, chunk, ko, f] with C = ko*128 + p
            w_qkv_loc = np.ascontiguousarray(
                w_flat.reshape(KO, 128, NCH, 128).transpose(1, 2, 0, 3)
            ).astype(NPBF)
            w_proj_loc = np.ascontiguousarray(
                W_proj[s * FV:(s + 1) * FV, :]).astype(NPBF)
            in_maps.append(
                {
                    "x_t": x_tg,
                    "w_qkv": w_qkv_loc,
                    "w_proj": w_proj_loc,
                    "sin_t": sin_t,
                    "cos_t": cos_t,
                    "ones_col": ones_col,
                    "ones_row": ones_row,
                    "tri": tri,
                }
            )

    trace = bool(int(os.environ.get("KERNEL_TRACE", "0")))
    if trace:
        _install_ntff_hook()
    nc = _get_nc()
    res = run_bass_kernel_spmd(
        nc, in_maps, core_ids=list(range(NCORES)), trace=trace
    )
    _CACHED["last_result"] = res

    out = np.zeros((B, T, C), dtype=np.float32)
    for g in range(BGROUPS):
        acc = np.zeros((B_LOC, C, T), dtype=np.float32)
        for s in range(HSHARDS):
            acc += res.results[g * HSHARDS + s]["out_t"].astype(np.float32)
        out[g * B_LOC:(g + 1) * B_LOC] = acc.transpose(0, 2, 1)
    return out
